# revision 1
# baseline (speedup 1.0000x reference)
"""DIEN forward-loss kernel for Trainium2, SPMD over 8 NeuronCores.

Sharding: data-parallel over batch (B=256 -> 32 rows/core), embedding table
replicated. Each core runs: embedding gather + max_norm renorm -> GRU (interest
extraction) -> aux BCE partial -> AUGRU (interest evolution, attention == 1)
-> AllGather(h, item, aux_sum) -> replicated final MLP with Dice batchnorm
(full-batch stats) + BCE -> identical scalar loss on every core.

All recurrence state is kept transposed ([D on partitions, batch on free]) so
the per-step matmuls need no transposes. x-side matmuls are chunked (8 steps,
N=256) and accumulated in PSUM; per-step h-side matmuls accumulate into the
same PSUM chunk so the sigmoid reads a single buffer.
"""
import numpy as np
import concourse.bass as bass
import concourse.bacc as bacc
import concourse.mybir as mybir
import concourse.tile as tile
from concourse.bass_utils import run_bass_kernel_spmd
from concourse.masks import make_identity

F32 = mybir.dt.float32
I32 = mybir.dt.int32
AF = mybir.ActivationFunctionType
OP = mybir.AluOpType

# problem constants (hardcoded; kernel.py must be self-contained)
B, L, D, NV = 256, 200, 128, 500000
NCORES = 8
BL = B // NCORES          # 32 batch rows per core
NT = L * BL               # 6400 (t,b) pairs per core
NTIL = NT // 128          # 50 gather tiles
CH = 8                    # recurrence chunk (timesteps per x-side matmul)
CW = CH * BL              # 256 columns per chunk
NCHUNK = L // CH          # 25
LAG = CH                  # AUGRU lags GRU by one chunk
EPS_BN = 1e-5
DICE_A = 0.1
ALPHA = 0.2
MAGIC = 0x5F3759DF


def _rsqrt(nc, pool, v, out, shape, iters=4):
    """out = 1/sqrt(v) elementwise on DVE only (no ACT tables).

    Quake seed via int bit-trick, then Newton iterations
    y <- y * (1.5 - 0.5 * v * y^2). v must be >= 0; v == 0 gives a large
    finite value (callers clamp with min()).
    """
    p, n = shape
    iv = out.bitcast(I32)
    nc.vector.tensor_scalar(
        out=iv, in0=v.bitcast(I32), scalar1=1, scalar2=None,
        op0=OP.arith_shift_right,
    )
    # magic - i  ==  (i xor -1) + (magic + 1)
    nc.vector.tensor_scalar(
        out=iv, in0=iv, scalar1=-1, scalar2=None,
        op0=OP.bitwise_xor,
    )
    nc.vector.tensor_scalar(
        out=iv, in0=iv, scalar1=MAGIC + 1, scalar2=None,
        op0=OP.add,
    )
    t = pool.tile([p, n], F32, tag="rsqrt_t")
    for _ in range(iters):
        nc.vector.tensor_tensor(out=t[:], in0=v, in1=out, op=OP.mult)
        nc.vector.tensor_tensor(out=t[:], in0=t[:], in1=out, op=OP.mult)
        nc.vector.tensor_scalar(
            out=t[:], in0=t[:], scalar1=-0.5, scalar2=1.5,
            op0=OP.mult, op1=OP.add,
        )
        nc.vector.tensor_tensor(out=out, in0=out, in1=t[:], op=OP.mult)


def build_bass(upto="full"):
    # upto: "A" (gather only), "G" (+GRU), "GA" (+AUGRU), "X" (+aux/gather),
    #       "full". Truncated builds write a debug value to out.
    nc = bacc.Bacc("TRN2", target_bir_lowering=False, num_devices=NCORES)

    # ---------------- kernel parameters ----------------
    emb = nc.declare_dram_parameter("emb", [NV, D], F32, isOutput=False)
    idx_h = nc.declare_dram_parameter("idx_h", [128, NTIL], I32, isOutput=False)
    y_h = nc.declare_dram_parameter("y_h", [128, NTIL], F32, isOutput=False)
    idx_t = nc.declare_dram_parameter("idx_t", [BL, 1], I32, isOutput=False)
    wihT = nc.declare_dram_parameter("wihT", [D, 3 * D], F32, isOutput=False)
    whhT = nc.declare_dram_parameter("whhT", [D, 3 * D], F32, isOutput=False)
    bias_gi = nc.declare_dram_parameter("bias_gi", [1, 2 * D], F32, isOutput=False)
    bihn = nc.declare_dram_parameter("bihn", [1, D], F32, isOutput=False)
    bhhn = nc.declare_dram_parameter("bhhn", [1, D], F32, isOutput=False)
    Wall = nc.declare_dram_parameter("Wall", [D, 3 * D], F32, isOutput=False)
    Uall = nc.declare_dram_parameter("Uall", [D, 3 * D], F32, isOutput=False)
    bias_ur = nc.declare_dram_parameter("bias_ur", [1, 2 * D], F32, isOutput=False)
    bh_aug = nc.declare_dram_parameter("bh_aug", [1, D], F32, isOutput=False)
    W1 = nc.declare_dram_parameter("W1", [2 * D, D], F32, isOutput=False)
    b1 = nc.declare_dram_parameter("b1", [1, D], F32, isOutput=False)
    W2 = nc.declare_dram_parameter("W2", [D, D // 2], F32, isOutput=False)
    b2 = nc.declare_dram_parameter("b2", [1, D // 2], F32, isOutput=False)
    Wf = nc.declare_dram_parameter("Wf", [D // 2, 1], F32, isOutput=False)
    bf = nc.declare_dram_parameter("bf", [1, 1], F32, isOutput=False)
    h0T = nc.declare_dram_parameter("h0T", [D, BL], F32, isOutput=False)
    y_t = nc.declare_dram_parameter("y_t", [1, B], F32, isOutput=False)
    out_p = nc.declare_dram_parameter("out", [1, 1], F32, isOutput=True)

    # internal DRAM for the collective
    ploc = nc.dram_tensor("ploc", [BL + 1, 2 * D + 1], F32)
    gall = nc.dram_tensor("gall", [NCORES * (BL + 1), 2 * D + 1], F32)

    with tile.TileContext(nc) as tc:
        with (
            tc.tile_pool(name="persist", bufs=1) as pp,
            tc.tile_pool(name="work", bufs=2) as wk,
            tc.tile_pool(name="ps_ck", bufs=3, space="PSUM") as pck,
            tc.tile_pool(name="ps_st", bufs=2, space="PSUM") as pst,
            tc.tile_pool(name="ps_g", bufs=1, space="PSUM") as psg,
        ):
            # ---------------- constants / weights to SBUF ----------------
            ident = pp.tile([128, 128], F32, tag="ident")
            make_identity(nc, ident[:])
            ones_col = pp.tile([128, 1], F32, tag="ones_col")
            nc.gpsimd.memset(ones_col[:], 1.0)
            zeros_b = pp.tile([128, BL], F32, tag="zeros_b")
            nc.gpsimd.memset(zeros_b[:], 0.0)

            def load(name_ap, shape, tag):
                t = pp.tile(shape, F32, tag=tag)
                nc.sync.dma_start(out=t[:], in_=name_ap[:])
                return t

            wihT_s = load(wihT, [D, 3 * D], "wihT")
            whhT_s = load(whhT, [D, 3 * D], "whhT")
            Wall_s = load(Wall, [D, 3 * D], "Wall")
            Uall_s = load(Uall, [D, 3 * D], "Uall")
            W1a_s = pp.tile([D, D], F32, tag="W1a")
            nc.sync.dma_start(out=W1a_s[:], in_=W1[0:D, :])
            W1b_s = pp.tile([D, D], F32, tag="W1b")
            nc.sync.dma_start(out=W1b_s[:], in_=W1[D:2 * D, :])
            b1_s = load(b1, [1, D], "b1")
            W2_s = load(W2, [D, D // 2], "W2")
            b2_s = load(b2, [1, D // 2], "b2")
            Wf_s = load(Wf, [D // 2, 1], "Wf")
            bf_s = load(bf, [1, 1], "bf")
            y_t_s = load(y_t, [1, B], "y_t")
            y_h_s = load(y_h, [128, NTIL], "y_h")
            hA = load(h0T, [D, BL], "hA")  # AUGRU state (in-place updated)

            idx_s = pp.tile([128, NTIL], I32, tag="idx_s")
            nc.sync.dma_start(out=idx_s[:], in_=idx_h[:])
            idx_t_s = pp.tile([BL, 1], I32, tag="idx_t_s")
            nc.sync.dma_start(out=idx_t_s[:], in_=idx_t[:])

            # bias APs for per-partition ACT bias: need [128, 1] views.
            # bias_* are [1, N] in SBUF -> we need them per-partition instead.
            # Load transposed copies via DMA from DRAM with AP rearrange.
            def load_col(src_ap, tag):
                t = pp.tile([D, 1], F32, tag=tag)
                nc.sync.dma_start(out=t[:], in_=src_ap.rearrange("o d -> d o"))
                return t

            bihn_c = load_col(bihn[:], "bihn_c")
            bhhn_c = load_col(bhhn[:], "bhhn_c")
            bh_aug_c = load_col(bh_aug[:], "bh_aug_c")
            br_c = load_col(bias_gi[0:1, 0:D], "br_c")
            bz_c = load_col(bias_gi[0:1, D:2 * D], "bz_c")
            bu_c = load_col(bias_ur[0:1, 0:D], "bu_c")
            bur_c = load_col(bias_ur[0:1, D:2 * D], "bur_c")

            # persistent big buffers
            ET = pp.tile([128, NT], F32, tag="ET")          # e^T  [d, (t,b)]
            outsT = pp.tile([128, NT], F32, tag="outsT")    # GRU outs^T
            s_all = pp.tile([128, NTIL], F32, tag="s_all")  # aux logits
            ss_all = pp.tile([128, NTIL], F32, tag="ss_all")
            erows = pp.tile([128, NT], F32, tag="erows")    # gathered rows

            # =========== Phase A: embedding gather + renorm + transpose ====
            for k in range(NTIL):
                sl = erows[:, 128 * k:128 * (k + 1)]
                nc.gpsimd.indirect_dma_start(
                    out=sl, out_offset=None, in_=emb[:],
                    in_offset=bass.IndirectOffsetOnAxis(ap=idx_s[:, k:k + 1], axis=0),
                )
                sq = wk.tile([128, 128], F32, tag="sq_scr")
                nc.scalar.activation(sq[:], sl, AF.Square,
                                     accum_out=ss_all[:, k:k + 1])
            scale = pp.tile([128, NTIL], F32, tag="scale")
            _rsqrt(nc, wk, ss_all[:], scale[:], [128, NTIL])
            nc.vector.tensor_scalar_min(out=scale[:], in0=scale[:], scalar1=1.0)
            for k in range(NTIL):
                sl = erows[:, 128 * k:128 * (k + 1)]
                nc.vector.tensor_scalar(
                    out=sl, in0=sl, scalar1=scale[:, k:k + 1], scalar2=None,
                    op0=OP.mult,
                )
                tp = psg.tile([128, 128], F32, tag="gram")
                nc.tensor.transpose(out=tp[:], in_=sl, identity=ident[:])
                nc.vector.tensor_copy(ET[:, 128 * k:128 * (k + 1)], tp[:])

            # target item: gather 32 rows + renorm (keep row layout)
            itemr = pp.tile([BL, D], F32, tag="itemr")
            nc.gpsimd.indirect_dma_start(
                out=itemr[:], out_offset=None, in_=emb[:],
                in_offset=bass.IndirectOffsetOnAxis(ap=idx_t_s[:, :1], axis=0),
            )
            sqt = wk.tile([BL, D], F32, tag="sqt")
            sst = wk.tile([BL, 1], F32, tag="sst")
            nc.scalar.activation(sqt[:], itemr[:], AF.Square, accum_out=sst[:])
            sct = wk.tile([BL, 1], F32, tag="sct")
            _rsqrt(nc, wk, sst[:], sct[:], [BL, 1])
            nc.vector.tensor_scalar_min(out=sct[:], in0=sct[:], scalar1=1.0)
            nc.vector.tensor_scalar(
                out=itemr[:], in0=itemr[:], scalar1=sct[:], scalar2=None,
                op0=OP.mult,
            )

            if upto == "A":
                dbg = wk.tile([1, 1], F32, tag="res")
                nc.vector.reduce_sum(out=dbg[:], in_=ET[0:1, 0:128],
                                     axis=mybir.AxisListType.X)
                nc.sync.dma_start(out=out_p[:], in_=dbg[:])

            # =========== Phase B/C/D: GRU + aux grams + AUGRU ==============
            gruRZ = None
            gruN = None
            augUR = None
            augH = None
            for slot in range(L + LAG if upto != "A" else 0):
                tg = slot
                ta = slot - LAG
                # ---- GRU x-side chunk: 3 matmuls + biased copies to SBUF ----
                if tg < L and tg % CH == 0:
                    ecols = ET[:, tg * BL: tg * BL + CW]
                    rz_sb = wk.tile([128, 2 * CW], F32, tag="g_rzck")
                    ckR = pck.tile([128, CW], F32, tag="ck")
                    nc.tensor.matmul(ckR[:], wihT_s[:, 0:D], ecols,
                                     start=True, stop=True)
                    nc.vector.tensor_scalar(
                        out=rz_sb[:, 0:CW], in0=ckR[:],
                        scalar1=br_c[:, 0:1], scalar2=None, op0=OP.add)
                    ckZ = pck.tile([128, CW], F32, tag="ck")
                    nc.tensor.matmul(ckZ[:], wihT_s[:, D:2 * D], ecols,
                                     start=True, stop=True)
                    nc.vector.tensor_scalar(
                        out=rz_sb[:, CW:2 * CW], in0=ckZ[:],
                        scalar1=bz_c[:, 0:1], scalar2=None, op0=OP.add)
                    ckN = pck.tile([128, CW], F32, tag="ck")
                    nc.tensor.matmul(ckN[:], wihT_s[:, 2 * D:3 * D], ecols,
                                     start=True, stop=True)
                    gin_sb = wk.tile([128, CW], F32, tag="g_nck")
                    nc.scalar.activation(gin_sb[:], ckN[:], AF.Copy)
                # ---- GRU step ----
                if tg < L:
                    o = tg % CH
                    h_prev = (zeros_b[:, 0:BL] if tg == 0
                              else outsT[:, (tg - 1) * BL: tg * BL])
                    hz = pst.tile([128, 3 * BL], F32, tag="ghz")
                    nc.tensor.matmul(hz[:, 2 * BL:3 * BL], whhT_s[:, 2 * D:3 * D],
                                     h_prev, start=True, stop=False)
                    nc.tensor.matmul(hz[:, 0:BL], whhT_s[:, 0:D], h_prev,
                                     start=False, stop=False)
                    nc.tensor.matmul(hz[:, BL:2 * BL], whhT_s[:, D:2 * D], h_prev,
                                     start=False, stop=True)
                    srz = wk.tile([128, 2 * BL], F32, tag="g_srz")
                    nc.vector.tensor_tensor(
                        out=srz[:, 0:BL], in0=hz[:, 0:BL],
                        in1=rz_sb[:, o * BL:(o + 1) * BL], op=OP.add)
                    nc.vector.tensor_tensor(
                        out=srz[:, BL:2 * BL], in0=hz[:, BL:2 * BL],
                        in1=rz_sb[:, CW + o * BL:CW + (o + 1) * BL], op=OP.add)
                    rz = wk.tile([128, 2 * BL], F32, tag="g_rz")
                    nc.scalar.activation(rz[:], srz[:], AF.Sigmoid)
                    t1 = wk.tile([128, BL], F32, tag="g_t1")
                    nc.vector.tensor_scalar(
                        out=t1[:], in0=hz[:, 2 * BL:3 * BL],
                        scalar1=bhhn_c[:, 0:1], scalar2=None, op0=OP.add)
                    nc.vector.tensor_tensor(out=t1[:], in0=t1[:],
                                            in1=rz[:, 0:BL], op=OP.mult)
                    t2 = wk.tile([128, BL], F32, tag="g_t2")
                    nc.vector.tensor_tensor(
                        out=t2[:], in0=t1[:],
                        in1=gin_sb[:, o * BL:(o + 1) * BL], op=OP.add)
                    nt = wk.tile([128, BL], F32, tag="g_n")
                    nc.scalar.activation(nt[:], t2[:], AF.Tanh,
                                         bias=bihn_c[:, 0:1])
                    d1 = wk.tile([128, BL], F32, tag="g_d1")
                    nc.vector.tensor_tensor(out=d1[:], in0=h_prev, in1=nt[:],
                                            op=OP.subtract)
                    d2 = wk.tile([128, BL], F32, tag="g_d2")
                    nc.vector.tensor_tensor(out=d2[:], in0=rz[:, BL:2 * BL],
                                            in1=d1[:], op=OP.mult)
                    nc.vector.tensor_tensor(
                        out=outsT[:, tg * BL:(tg + 1) * BL], in0=nt[:],
                        in1=d2[:], op=OP.add)
                # ---- aux gram: diag(outs_blk^T @ E_blk) ----
                if tg < L and tg % 4 == 3:
                    blk = tg // 4
                    gps = psg.tile([128, 128], F32, tag="gram")
                    nc.tensor.matmul(
                        gps[:], outsT[:, 128 * blk:128 * (blk + 1)],
                        ET[:, 128 * blk:128 * (blk + 1)], start=True, stop=True)
                    gsc = wk.tile([128, 128], F32, tag="gram_scr")
                    nc.vector.tensor_tensor(out=gsc[:], in0=gps[:],
                                            in1=ident[:], op=OP.mult)
                    nc.vector.reduce_sum(out=s_all[:, blk:blk + 1], in_=gsc[:],
                                         axis=mybir.AxisListType.X)
                if upto == "G":
                    continue
                # ---- AUGRU x-side chunk ----
                if 0 <= ta < L and ta % CH == 0:
                    ocols = outsT[:, ta * BL: ta * BL + CW]
                    ur_sb = wk.tile([128, 2 * CW], F32, tag="a_urck")
                    ckU = pck.tile([128, CW], F32, tag="ck")
                    nc.tensor.matmul(ckU[:], Wall_s[:, 0:D], ocols,
                                     start=True, stop=True)
                    nc.vector.tensor_scalar(
                        out=ur_sb[:, 0:CW], in0=ckU[:],
                        scalar1=bu_c[:, 0:1], scalar2=None, op0=OP.add)
                    ckR2 = pck.tile([128, CW], F32, tag="ck")
                    nc.tensor.matmul(ckR2[:], Wall_s[:, D:2 * D], ocols,
                                     start=True, stop=True)
                    nc.vector.tensor_scalar(
                        out=ur_sb[:, CW:2 * CW], in0=ckR2[:],
                        scalar1=bur_c[:, 0:1], scalar2=None, op0=OP.add)
                    ckH = pck.tile([128, CW], F32, tag="ck")
                    nc.tensor.matmul(ckH[:], Wall_s[:, 2 * D:3 * D], ocols,
                                     start=True, stop=True)
                    xh_sb = wk.tile([128, CW], F32, tag="a_hck")
                    nc.scalar.activation(xh_sb[:], ckH[:], AF.Copy)
                # ---- AUGRU step (attention weight == 1) ----
                if 0 <= ta < L:
                    o2 = ta % CH
                    hz2 = pst.tile([128, 3 * BL], F32, tag="ahz")
                    nc.tensor.matmul(hz2[:, 2 * BL:3 * BL], Uall_s[:, 2 * D:3 * D],
                                     hA[:], start=True, stop=False)
                    nc.tensor.matmul(hz2[:, 0:BL], Uall_s[:, 0:D], hA[:],
                                     start=False, stop=False)
                    nc.tensor.matmul(hz2[:, BL:2 * BL], Uall_s[:, D:2 * D], hA[:],
                                     start=False, stop=True)
                    sur = wk.tile([128, 2 * BL], F32, tag="a_sur")
                    nc.vector.tensor_tensor(
                        out=sur[:, 0:BL], in0=hz2[:, 0:BL],
                        in1=ur_sb[:, o2 * BL:(o2 + 1) * BL], op=OP.add)
                    nc.vector.tensor_tensor(
                        out=sur[:, BL:2 * BL], in0=hz2[:, BL:2 * BL],
                        in1=ur_sb[:, CW + o2 * BL:CW + (o2 + 1) * BL], op=OP.add)
                    ur = wk.tile([128, 2 * BL], F32, tag="a_ur")
                    nc.scalar.activation(ur[:], sur[:], AF.Sigmoid)
                    t1a = wk.tile([128, BL], F32, tag="a_t1")
                    nc.vector.tensor_tensor(out=t1a[:], in0=ur[:, BL:2 * BL],
                                            in1=hz2[:, 2 * BL:3 * BL], op=OP.mult)
                    t2a = wk.tile([128, BL], F32, tag="a_t2")
                    nc.vector.tensor_tensor(
                        out=t2a[:], in0=t1a[:],
                        in1=xh_sb[:, o2 * BL:(o2 + 1) * BL], op=OP.add)
                    hht = wk.tile([128, BL], F32, tag="a_hh")
                    nc.scalar.activation(hht[:], t2a[:], AF.Tanh,
                                         bias=bh_aug_c[:, 0:1])
                    d1a = wk.tile([128, BL], F32, tag="a_d1")
                    nc.vector.tensor_tensor(out=d1a[:], in0=hht[:], in1=hA[:],
                                            op=OP.subtract)
                    d2a = wk.tile([128, BL], F32, tag="a_d2")
                    nc.vector.tensor_tensor(out=d2a[:], in0=ur[:, 0:BL],
                                            in1=d1a[:], op=OP.mult)
                    nc.vector.tensor_tensor(out=hA[:], in0=hA[:], in1=d2a[:],
                                            op=OP.add)

            if upto == "G":
                dbg = wk.tile([1, 1], F32, tag="res")
                nc.vector.reduce_sum(out=dbg[:], in_=outsT[0:1, NT - 128:NT],
                                     axis=mybir.AxisListType.X)
                nc.sync.dma_start(out=out_p[:], in_=dbg[:])
            if upto == "GA":
                dbg = wk.tile([1, 1], F32, tag="res")
                nc.vector.reduce_sum(out=dbg[:], in_=hA[0:1, :],
                                     axis=mybir.AxisListType.X)
                nc.sync.dma_start(out=out_p[:], in_=dbg[:])
            do_aux = upto in ("X", "full")

            if do_aux:
                # =========== aux BCE partial sum (Exp/Ln table) ================
                ebuf = pp.tile([128, NTIL], F32, tag="ebuf")
                nc.scalar.activation(ebuf[:], s_all[:], AF.Exp)
                nc.vector.tensor_scalar_add(out=ebuf[:], in0=ebuf[:], scalar1=1.0)
                sp = pp.tile([128, NTIL], F32, tag="sp")
                nc.scalar.activation(sp[:], ebuf[:], AF.Ln)
                spm = pp.tile([128, NTIL], F32, tag="spm")
                nc.vector.tensor_tensor(out=spm[:], in0=sp[:], in1=s_all[:],
                                        op=OP.subtract)
                nc.vector.tensor_scalar_min(out=spm[:], in0=spm[:], scalar1=100.0)
                nc.vector.tensor_scalar_min(out=sp[:], in0=sp[:], scalar1=100.0)
                # loss_i = sp + y*(spm - sp)
                nc.vector.tensor_tensor(out=spm[:], in0=spm[:], in1=sp[:],
                                        op=OP.subtract)
                nc.vector.tensor_tensor(out=spm[:], in0=y_h_s[:], in1=spm[:],
                                        op=OP.mult)
                nc.vector.tensor_tensor(out=sp[:], in0=sp[:], in1=spm[:], op=OP.add)
                rsum = wk.tile([128, 1], F32, tag="rsum")
                nc.vector.reduce_sum(out=rsum[:], in_=sp[:],
                                     axis=mybir.AxisListType.X)
                aux_ps = psg.tile([1, 1], F32, tag="gram")
                nc.tensor.matmul(aux_ps[:], rsum[:], ones_col[:, 0:1],
                                 start=True, stop=True)
                aux_sc = wk.tile([1, 1], F32, tag="aux_sc")
                nc.vector.tensor_copy(aux_sc[:], aux_ps[:])

                # =========== pack + AllGather ==================================
                # hA -> rows [BL, D]
                hrow_ps = psg.tile([BL, D], F32, tag="gram")
                nc.tensor.transpose(out=hrow_ps[:], in_=hA[:], identity=ident[:])
                stage = pp.tile([BL + 1, 2 * D + 1], F32, tag="stage")
                nc.gpsimd.memset(stage[:], 0.0)
                nc.vector.tensor_copy(stage[0:BL, 0:D], hrow_ps[:])
                nc.vector.tensor_copy(stage[0:BL, D:2 * D], itemr[:])
                nc.vector.tensor_copy(stage[BL:BL + 1, 0:1], aux_sc[:])
                nc.sync.dma_start(out=ploc[:], in_=stage[:])
                nc.gpsimd.collective_compute(
                    "AllGather", OP.bypass,
                    replica_groups=[list(range(NCORES))],
                    ins=[ploc[:]], outs=[gall[:]],
                )

            if upto == "X":
                dbg = wk.tile([1, 1], F32, tag="res")
                nc.vector.tensor_copy(dbg[:], aux_sc[:])
                nc.sync.dma_start(out=out_p[:], in_=dbg[:])

            if upto == "full":
                # =========== replicated final MLP ==============================
                PW = 2 * D + 1  # gall row width
                hT_all = pp.tile([D, B], F32, tag="hT_all")
                iT_all = pp.tile([D, B], F32, tag="iT_all")
                for half in range(2):
                    hr = wk.tile([128, 128], F32, tag="hr_half")
                    ir_ = wk.tile([128, 128], F32, tag="ir_half")
                    for j in range(4):
                        c = 4 * half + j
                        r0 = (BL + 1) * c
                        nc.sync.dma_start(out=hr[BL * j:BL * (j + 1), :],
                                          in_=gall[r0:r0 + BL, 0:D])
                        nc.sync.dma_start(out=ir_[BL * j:BL * (j + 1), :],
                                          in_=gall[r0:r0 + BL, D:2 * D])
                    tp = psg.tile([128, 128], F32, tag="gram")
                    nc.tensor.transpose(out=tp[:], in_=hr[:], identity=ident[:])
                    nc.vector.tensor_copy(hT_all[:, 128 * half:128 * (half + 1)],
                                          tp[:])
                    tp2 = psg.tile([128, 128], F32, tag="gram")
                    nc.tensor.transpose(out=tp2[:], in_=ir_[:], identity=ident[:])
                    nc.vector.tensor_copy(iT_all[:, 128 * half:128 * (half + 1)],
                                          tp2[:])
                aux8 = wk.tile([1, NCORES], F32, tag="aux8")
                for c in range(NCORES):
                    nc.sync.dma_start(out=aux8[0:1, c:c + 1],
                                      in_=gall[(BL + 1) * c + BL:(BL + 1) * c + BL + 1,
                                               0:1])
                aux_tot = wk.tile([1, 1], F32, tag="aux_tot")
                nc.vector.reduce_sum(out=aux_tot[:], in_=aux8[:],
                                     axis=mybir.AxisListType.X)

                ones_b = pp.tile([1, B], F32, tag="ones_b")
                nc.gpsimd.memset(ones_b[:], 1.0)

                def dice(z_ps, pdim):
                    """Dice on z (psum [pdim, B], batch on free axis).
                    Returns SBUF tile z*(0.1+0.9*sigmoid((z-mu)/std))."""
                    m = wk.tile([pdim, 1], F32, tag="dice_m")
                    nc.vector.reduce_sum(out=m[:], in_=z_ps[:],
                                         axis=mybir.AxisListType.X)
                    nc.vector.tensor_scalar_mul(out=m[:], in0=m[:], scalar1=1.0 / B)
                    xc = wk.tile([pdim, B], F32, tag="dice_xc")
                    nc.vector.tensor_scalar(out=xc[:], in0=z_ps[:], scalar1=m[:],
                                            scalar2=None, op0=OP.subtract)
                    sq2 = wk.tile([pdim, B], F32, tag="dice_sq")
                    vs = wk.tile([pdim, 1], F32, tag="dice_vs")
                    nc.scalar.activation(sq2[:], xc[:], AF.Square, accum_out=vs[:])
                    nc.vector.tensor_scalar(out=vs[:], in0=vs[:], scalar1=1.0 / B,
                                            scalar2=EPS_BN, op0=OP.mult, op1=OP.add)
                    inv = wk.tile([pdim, 1], F32, tag="dice_inv")
                    _rsqrt(nc, wk, vs[:], inv[:], [pdim, 1])
                    pr = wk.tile([pdim, B], F32, tag="dice_p")
                    nc.scalar.activation(pr[:], xc[:], AF.Sigmoid, scale=inv[:, 0:1])
                    nc.vector.tensor_scalar(out=pr[:], in0=pr[:], scalar1=1 - DICE_A,
                                            scalar2=DICE_A, op0=OP.mult, op1=OP.add)
                    zd = wk.tile([pdim, B], F32, tag="dice_zd")
                    nc.vector.tensor_tensor(out=zd[:], in0=z_ps[:], in1=pr[:],
                                            op=OP.mult)
                    return zd

                z1_ps = pck.tile([128, B], F32, tag="ck")
                nc.tensor.matmul(z1_ps[:], W1a_s[:], hT_all[:],
                                 start=True, stop=False)
                nc.tensor.matmul(z1_ps[:], W1b_s[:], iT_all[:],
                                 start=False, stop=False)
                nc.tensor.matmul(z1_ps[:], b1_s[0:1, :], ones_b[0:1, :],
                                 start=False, stop=True)
                z1d = dice(z1_ps, 128)

                z2_ps = pck.tile([D // 2, B], F32, tag="ck")
                nc.tensor.matmul(z2_ps[:], W2_s[:, :], z1d[:],
                                 start=True, stop=False)
                nc.tensor.matmul(z2_ps[:], b2_s[0:1, :], ones_b[0:1, :],
                                 start=False, stop=True)
                z2d = dice(z2_ps, D // 2)

                s_ps = pck.tile([1, B], F32, tag="ck")
                nc.tensor.matmul(s_ps[:], Wf_s[:, 0:1], z2d[:],
                                 start=True, stop=False)
                nc.tensor.matmul(s_ps[:], bf_s[0:1, 0:1], ones_b[0:1, :],
                                 start=False, stop=True)
                s_sb = wk.tile([1, B], F32, tag="s_sb")
                nc.vector.tensor_copy(s_sb[:], s_ps[:])

                # rec BCE over the full batch (replicated on every core)
                e2 = wk.tile([1, B], F32, tag="e2")
                nc.scalar.activation(e2[:], s_sb[:], AF.Exp)
                nc.vector.tensor_scalar_add(out=e2[:], in0=e2[:], scalar1=1.0)
                sp2 = wk.tile([1, B], F32, tag="sp2")
                nc.scalar.activation(sp2[:], e2[:], AF.Ln)
                spm2 = wk.tile([1, B], F32, tag="spm2")
                nc.vector.tensor_tensor(out=spm2[:], in0=sp2[:], in1=s_sb[:],
                                        op=OP.subtract)
                nc.vector.tensor_scalar_min(out=spm2[:], in0=spm2[:], scalar1=100.0)
                nc.vector.tensor_scalar_min(out=sp2[:], in0=sp2[:], scalar1=100.0)
                nc.vector.tensor_tensor(out=spm2[:], in0=spm2[:], in1=sp2[:],
                                        op=OP.subtract)
                nc.vector.tensor_tensor(out=spm2[:], in0=y_t_s[:], in1=spm2[:],
                                        op=OP.mult)
                nc.vector.tensor_tensor(out=sp2[:], in0=sp2[:], in1=spm2[:],
                                        op=OP.add)
                rec_sum = wk.tile([1, 1], F32, tag="rec_sum")
                nc.vector.reduce_sum(out=rec_sum[:], in_=sp2[:],
                                     axis=mybir.AxisListType.X)

                nc.vector.tensor_scalar_mul(out=aux_tot[:], in0=aux_tot[:],
                                            scalar1=ALPHA / (B * L))
                nc.vector.tensor_scalar_mul(out=rec_sum[:], in0=rec_sum[:],
                                            scalar1=1.0 / B)
                res = wk.tile([1, 1], F32, tag="res")
                nc.vector.tensor_tensor(out=res[:], in0=aux_tot[:], in1=rec_sum[:],
                                        op=OP.add)
                nc.sync.dma_start(out=out_p[:], in_=res[:])
    nc.compile()
    return nc


_NC_CACHE = None


def _get_nc():
    global _NC_CACHE
    if _NC_CACHE is None:
        import os
        _NC_CACHE = build_bass(os.environ.get("KERNEL_UPTO", "full"))
    return _NC_CACHE


def _prep_inputs(inputs):
    """Build the 8 per-core input maps from the full problem inputs."""
    f32 = np.float32
    emb = np.ascontiguousarray(inputs["emb"], dtype=f32)
    seqs = np.asarray(inputs["history_seqs"])          # [B, L] int32
    labs = np.asarray(inputs["history_labels"])        # [B, L, 1] int32
    tgt = np.asarray(inputs["target_item"])            # [B] int32
    tl = np.asarray(inputs["target_label"]).astype(f32)  # [B]

    w_ih = np.asarray(inputs["w_ih"], dtype=f32)
    w_hh = np.asarray(inputs["w_hh"], dtype=f32)
    b_ih = np.asarray(inputs["b_ih"], dtype=f32)
    b_hh = np.asarray(inputs["b_hh"], dtype=f32)
    wihT = np.ascontiguousarray(w_ih.T)
    whhT = np.ascontiguousarray(w_hh.T)
    bias_gi = (b_ih[:2 * D] + b_hh[:2 * D]).reshape(1, 2 * D)
    bihn = b_ih[2 * D:].reshape(1, D)
    bhhn = b_hh[2 * D:].reshape(1, D)

    Wall = np.ascontiguousarray(
        np.concatenate([inputs["Wu"], inputs["Wr"], inputs["Wh"]], axis=1),
        dtype=f32)
    Uall = np.ascontiguousarray(
        np.concatenate([inputs["Uu"], inputs["Ur"], inputs["Uh"]], axis=1),
        dtype=f32)
    bias_ur = np.concatenate(
        [np.asarray(inputs["bu"], dtype=f32).reshape(-1),
         np.asarray(inputs["br"], dtype=f32).reshape(-1)]).reshape(1, 2 * D)
    bh_aug = np.asarray(inputs["bh"], dtype=f32).reshape(1, D)

    W1 = np.ascontiguousarray(inputs["W1"], dtype=f32)
    b1 = np.asarray(inputs["b1"], dtype=f32).reshape(1, D)
    W2 = np.ascontiguousarray(inputs["W2"], dtype=f32)
    b2 = np.asarray(inputs["b2"], dtype=f32).reshape(1, D // 2)
    Wf = np.ascontiguousarray(inputs["Wf"], dtype=f32)
    bf = np.asarray(inputs["bf"], dtype=f32).reshape(1, 1)
    h0 = np.asarray(inputs["h0"], dtype=f32)
    y_t_full = tl.reshape(1, B)

    shared = dict(emb=emb, wihT=wihT, whhT=whhT, bias_gi=bias_gi, bihn=bihn,
                  bhhn=bhhn, Wall=Wall, Uall=Uall, bias_ur=bias_ur,
                  bh_aug=bh_aug, W1=W1, b1=b1, W2=W2, b2=b2, Wf=Wf, bf=bf,
                  y_t=y_t_full)
    in_maps = []
    for c in range(NCORES):
        sl = slice(c * BL, (c + 1) * BL)
        # t-major flattening: n = t*BL + b  -> [128, NTIL] with n = 128k + p
        idx_f = np.ascontiguousarray(seqs[sl].T).reshape(-1)      # [NT]
        idx_h = np.ascontiguousarray(
            idx_f.reshape(NTIL, 128).T).astype(np.int32)
        y_f = np.ascontiguousarray(labs[sl, :, 0].T).reshape(-1).astype(f32)
        y_h = np.ascontiguousarray(y_f.reshape(NTIL, 128).T)
        h0T = np.ascontiguousarray(h0[sl].T)
        idx_tc = tgt[sl].reshape(BL, 1).astype(np.int32)
        m = dict(shared)
        m.update(idx_h=idx_h, y_h=y_h, idx_t=idx_tc, h0T=h0T)
        in_maps.append(m)
    return in_maps


def kernel(**inputs) -> np.ndarray:
    nc = _get_nc()
    in_maps = _prep_inputs(inputs)
    res = run_bass_kernel_spmd(nc, in_maps, core_ids=list(range(NCORES)))
    out = np.asarray(res.results[0]["out"], dtype=np.float32)
    return out.reshape(())

